# revision 1
# baseline (speedup 1.0000x reference)
"""MoE routing layer on 8 Trainium2 NeuronCores (data-parallel over batch).

Per core (4 samples):
  routing MLP -> cosine sim vs embeddings -> softmax weights wf[4,10]
  w_eff[b] = sum_n wf[b,n] * conv_w[n]  (conv is linear in weights ->
  10x fewer conv FLOPs than materializing all expert convs)
  out[b] = conv2d(x[b], w_eff[b]) + b_eff[b]

Conv is 9 shifted matmuls over the flat 58-wide grid (tap = constant
free-dim offset); two samples run concurrently on the PE array via
row tiling (partitions 0-63 / 64-127), fp32r for full-rate streaming.
"""
import sys

sys.path.insert(0, "/opt/trn_rl_repo")

import numpy as np

import concourse.bass as bass
import concourse.mybir as mybir
from concourse.masks import make_identity
from concourse.tile import TileContext

F32 = mybir.dt.float32
F32R = mybir.dt.float32r
AF = mybir.ActivationFunctionType
ALU = mybir.AluOpType
AX = mybir.AxisListType

NCORES = 8
BLOC = 4           # samples per core
CIN = 64
COUT = 64
H = W = 58
HW = H * W         # 3364
OH = OW = 56
NB = 10            # experts
EDIM = 64
RSIZE = 512
HID = 128
NTAP = 9
CHUNK_ROWS = 8
NCHUNK = 7         # 7*8 = 56 output rows
NFREE = CHUNK_ROWS * W  # 464 <= 512 (one PSUM bank)
TAP_OFF = [dy * W + dx for dy in range(3) for dx in range(3)]
PAIRED = True      # 2-sample row-tiled PE packing
CONV_DT = F32R


def fix_sync_waits(nc, cap=2):
    """This walrus build allows at most `cap` sem waits per instruction.
    Splice same-engine NoOps carrying the excess waits right before any
    over-subscribed instruction (waits happen earlier => same semantics)."""
    uid = [0]
    for f in nc.m.functions:
        for blk in f.blocks:
            insts = blk.instructions  # live list
            i = 0
            while i < len(insts):
                inst = insts[i]
                si = inst.sync_info
                waits = list(si.on_wait) if si and si.on_wait else []
                icap = 1
                if len(waits) <= icap:
                    i += 1
                    continue
                keep, excess = waits[-icap:], waits[:-icap]
                for k in range(0, len(excess), icap):
                    nop = mybir.InstNoOp(
                        name=f"{inst.name}-wsplit{uid[0]}", ins=[], outs=[]
                    )
                    uid[0] += 1
                    nop.engine = inst.engine
                    nop.sync_info = mybir.SyncInfo(
                        on_wait=excess[k : k + icap], on_update=[]
                    )
                    nc.register_instruction(nop, overwrite=True)
                    insts.insert(i, nop)
                    i += 1
                inst.sync_info = mybir.SyncInfo(
                    on_wait=keep,
                    on_update=list(si.on_update) if si and si.on_update else [],
                )
                i += 1


def build():
    nc = bass.Bass(num_swdge_queues=4)
    x = nc.dram_tensor("x", [BLOC, CIN, H, W], F32, kind="ExternalInput")
    rv = nc.dram_tensor("rv", [BLOC, RSIZE], F32, kind="ExternalInput")
    w1 = nc.dram_tensor("w1", [RSIZE, HID], F32, kind="ExternalInput")
    bias1 = nc.dram_tensor("bias1", [HID, 1], F32, kind="ExternalInput")
    w2 = nc.dram_tensor("w2", [HID, EDIM], F32, kind="ExternalInput")
    bias2 = nc.dram_tensor("bias2", [EDIM, 1], F32, kind="ExternalInput")
    emb = nc.dram_tensor("emb", [NB, EDIM], F32, kind="ExternalInput")
    cwp = nc.dram_tensor("cwp", [CIN, NB, NTAP, COUT], F32, kind="ExternalInput")
    cb = nc.dram_tensor("cb", [NB, COUT], F32, kind="ExternalInput")
    sel = nc.dram_tensor("sel", [2, BLOC, 128], F32, kind="ExternalInput")
    identin = nc.dram_tensor("identin", [128, 128], F32, kind="ExternalInput")
    out = nc.dram_tensor("out", [BLOC, COUT, OH, OW], F32, kind="ExternalOutput")

    with TileContext(nc) as tc:
        with (
            tc.tile_pool(name="consts", bufs=1) as consts,
            tc.tile_pool(name="work", bufs=2) as work,
            tc.tile_pool(name="stage", bufs=4) as stage,
            tc.tile_pool(name="ps", bufs=2, space="PSUM") as pspool,
            tc.tile_pool(name="psconv", bufs=2, space="PSUM") as psconv,
        ):
            # ---------- inputs / constants into SBUF ----------
            ident = consts.tile([128, 128], F32, tag="ident")
            nc.sync.dma_start(out=ident[:], in_=identin[:])
            ones64 = consts.tile([EDIM, 1], F32, tag="ones64")
            nc.vector.memset(ones64[:], 1.0)

            rvsb = consts.tile([BLOC, RSIZE], F32, tag="rvsb")
            nc.sync.dma_start(out=rvsb[:], in_=rv[:])
            w1sb = consts.tile([128, 4, HID], F32, tag="w1sb")
            nc.sync.dma_start(
                out=w1sb[:], in_=w1[:].rearrange("(c k) m -> k c m", k=128)
            )
            w2sb = consts.tile([HID, EDIM], F32, tag="w2sb")
            nc.sync.dma_start(out=w2sb[:], in_=w2[:])
            b1sb = consts.tile([HID, 1], F32, tag="b1sb")
            nc.sync.dma_start(out=b1sb[:], in_=bias1[:])
            b2sb = consts.tile([EDIM, 1], F32, tag="b2sb")
            nc.sync.dma_start(out=b2sb[:], in_=bias2[:])
            embsb = consts.tile([NB, EDIM], F32, tag="embsb")
            nc.sync.dma_start(out=embsb[:], in_=emb[:])
            cbsb = consts.tile([NB, COUT], F32, tag="cbsb")
            nc.sync.dma_start(out=cbsb[:], in_=cb[:])
            selsb = consts.tile([BLOC, 2, 128], F32, tag="selsb")
            nc.sync.dma_start(out=selsb[:], in_=sel[:].rearrange("j b p -> b j p"))

            cwp2 = consts.tile([128, NB, NTAP, COUT], F32, tag="cwp2")
            nc.sync.dma_start(out=cwp2[0:64], in_=cwp[:])
            nc.sync.dma_start(out=cwp2[64:128], in_=cwp[:])

            xt = []
            for j in range(2):
                t = consts.tile([128, HW + 4], CONV_DT, tag=f"xt{j}")
                nc.vector.memset(t[:, HW : HW + 4].bitcast(F32), 0.0)
                nc.gpsimd.dma_start(
                    out=t[0:64, 0:HW], in_=x[2 * j].rearrange("c h w -> c (h w)")
                )
                nc.gpsimd.dma_start(
                    out=t[64:128, 0:HW],
                    in_=x[2 * j + 1].rearrange("c h w -> c (h w)"),
                )
                xt.append(t)

            # ---------- routing MLP ----------
            # rv [4, 512] -> rvT [128, 4(chunk), 4(sample)] via PE transposes
            rvT = work.tile([128, 4, BLOC], F32, tag="rvT")
            for c in range(4):
                pst = pspool.tile([128, BLOC], F32, tag="small")
                nc.tensor.transpose(
                    pst[:], rvsb[:, c * 128 : (c + 1) * 128], ident[0:BLOC, 0:BLOC]
                )
                nc.scalar.copy(out=rvT[:, c, :], in_=pst[:])
            h1 = pspool.tile([HID, BLOC], F32, tag="small")
            for c in range(4):
                nc.tensor.matmul(
                    h1[:], w1sb[:, c, :], rvT[:, c, :], start=(c == 0), stop=(c == 3)
                )
            h1r = work.tile([HID, BLOC], F32, tag="h1r")
            nc.scalar.activation(
                out=h1r[:], in_=h1[:], func=AF.Relu, bias=b1sb[:], scale=1.0
            )
            rps = pspool.tile([EDIM, BLOC], F32, tag="small")
            nc.tensor.matmul(rps[:], w2sb[:], h1r[:], start=True, stop=True)
            rsb = work.tile([EDIM, BLOC], F32, tag="rsb")
            nc.scalar.activation(
                out=rsb[:], in_=rps[:], func=AF.Identity, bias=b2sb[:], scale=1.0
            )

            # ---------- cosine similarity ----------
            rsq = work.tile([EDIM, BLOC], F32, tag="rsq")
            nc.vector.tensor_mul(rsq[:], rsb[:], rsb[:])
            nsq = pspool.tile([BLOC, 1], F32, tag="small")
            nc.tensor.matmul(nsq[:], rsq[:], ones64[:], start=True, stop=True)
            rln = work.tile([BLOC, 1], F32, tag="rln")
            nc.scalar.activation(out=rln[:], in_=nsq[:], func=AF.Ln)
            rinv = work.tile([BLOC, 1], F32, tag="rinv")
            nc.scalar.activation(out=rinv[:], in_=rln[:], func=AF.Exp, scale=-0.5)

            esq = work.tile([NB, EDIM], F32, tag="esq")
            nc.vector.tensor_mul(esq[:], embsb[:], embsb[:])
            ensq = work.tile([NB, 1], F32, tag="ensq")
            nc.vector.tensor_reduce(ensq[:], esq[:], axis=AX.X, op=ALU.add)
            eln = work.tile([NB, 1], F32, tag="eln")
            nc.scalar.activation(out=eln[:], in_=ensq[:], func=AF.Ln)
            einv = work.tile([NB, 1], F32, tag="einv")
            nc.scalar.activation(out=einv[:], in_=eln[:], func=AF.Exp, scale=-0.5)
            embn = work.tile([NB, EDIM], F32, tag="embn")
            nc.vector.tensor_scalar_mul(out=embn[:], in0=embsb[:], scalar1=einv[:])
            embnT_ps = pspool.tile([EDIM, NB], F32, tag="small")
            nc.tensor.transpose(embnT_ps[:], embn[:], ident[0:NB, 0:NB])
            embnT = work.tile([EDIM, NB], F32, tag="embnT")
            nc.scalar.copy(out=embnT[:], in_=embnT_ps[:])

            simps = pspool.tile([BLOC, NB], F32, tag="small")
            nc.tensor.matmul(simps[:], rsb[:], embnT[:], start=True, stop=True)
            sim = work.tile([BLOC, NB], F32, tag="sim")
            nc.vector.tensor_scalar_mul(out=sim[:], in0=simps[:], scalar1=rinv[:])

            # ---------- softmax ----------
            mx = work.tile([BLOC, 1], F32, tag="mx")
            nc.vector.tensor_reduce(mx[:], sim[:], axis=AX.X, op=ALU.max)
            negmx = work.tile([BLOC, 1], F32, tag="negmx")
            nc.vector.tensor_scalar_mul(out=negmx[:], in0=mx[:], scalar1=-1.0)
            ex = work.tile([BLOC, NB], F32, tag="ex")
            nc.scalar.activation(
                out=ex[:], in_=sim[:], func=AF.Exp, bias=negmx[:], scale=1.0
            )
            s = work.tile([BLOC, 1], F32, tag="s")
            nc.vector.tensor_reduce(s[:], ex[:], axis=AX.X, op=ALU.add)
            sinv = work.tile([BLOC, 1], F32, tag="sinv")
            nc.vector.reciprocal(sinv[:], s[:])
            wf = work.tile([BLOC, NB], F32, tag="wf")
            nc.vector.tensor_scalar_mul(out=wf[:], in0=ex[:], scalar1=sinv[:])

            # ---------- effective conv bias ----------
            wfT_ps = pspool.tile([NB, BLOC], F32, tag="small")
            nc.tensor.transpose(wfT_ps[:], wf[:], ident[0:BLOC, 0:BLOC])
            wfT = work.tile([NB, BLOC], F32, tag="wfT")
            nc.scalar.copy(out=wfT[:], in_=wfT_ps[:])
            beff_ps = pspool.tile([COUT, BLOC], F32, tag="small")
            nc.tensor.matmul(beff_ps[:], cbsb[:], wfT[:], start=True, stop=True)
            beff = work.tile([COUT, BLOC], F32, tag="beff")
            nc.scalar.copy(out=beff[:], in_=beff_ps[:])

            # ---------- PE warmup: keep HAM busy until conv starts ----------
            warm_ps = pspool.tile([128, 512], F32, tag="warm")
            wl = ident[:].bitcast(mybir.dt.bfloat16)[:, 0:128]
            wr = w1sb[:].rearrange("p c m -> p (c m)").bitcast(mybir.dt.bfloat16)[:, 0:512]
            for _ in range(22):
                nc.tensor.matmul(warm_ps[:], wl, wr, start=True, stop=True)
            warm_sink = work.tile([1, 1], F32, tag="warm_sink")
            nc.scalar.copy(out=warm_sink[:], in_=warm_ps[0:1, 0:1])

            # ---------- both pairs: weights broadcast + w_eff first ----------
            weffs = []
            for j in range(2):
                wfbc_ps = pspool.tile([128, NB], F32, tag="small")
                nc.tensor.matmul(
                    wfbc_ps[:], selsb[:, j, :], wf[:], start=True, stop=True
                )
                wfbc = work.tile([128, NB], F32, tag=f"wfbc{j}")
                nc.scalar.copy(out=wfbc[:], in_=wfbc_ps[:])

                weff = work.tile([128, NTAP, COUT], CONV_DT, tag=f"weff{j}")
                for lo, hi in ((0, 5), (5, NTAP)):
                    nc.vector.tensor_scalar_mul(
                        out=weff[:, lo:hi], in0=cwp2[:, 0, lo:hi], scalar1=wfbc[:, 0:1]
                    )
                    for n in range(1, NB):
                        nc.vector.scalar_tensor_tensor(
                            out=weff[:, lo:hi],
                            in0=cwp2[:, n, lo:hi],
                            scalar=wfbc[:, n : n + 1],
                            in1=weff[:, lo:hi],
                            op0=ALU.mult,
                            op1=ALU.add,
                        )
                weffs.append(weff)

            # ---------- PE warmup: keep HAM busy until conv starts ----------
            warm_ps = pspool.tile([128, 512], F32, tag="warm")
            wl = ident[:].bitcast(mybir.dt.bfloat16)[:, 0:128]
            wr = w1sb[:].rearrange("p c m -> p (c m)").bitcast(mybir.dt.bfloat16)[:, 0:512]
            for _ in range(22):
                nc.tensor.matmul(warm_ps[:], wl, wr, start=True, stop=True)
            warm_sink = work.tile([1, 1], F32, tag="warm_sink")
            nc.scalar.copy(out=warm_sink[:], in_=warm_ps[0:1, 0:1])

            # ---------- conv ----------
            for j in range(2):
                weff = weffs[j]
                for ch in range(NCHUNK):
                    h0 = ch * CHUNK_ROWS
                    psA = psconv.tile([COUT, NFREE], F32, tag="psA")
                    psB = psconv.tile([COUT, NFREE], F32, tag="psB")
                    for t in range(NTAP):
                        off = h0 * W + TAP_OFF[t]
                        nc.tensor.matmul(
                            psA[:],
                            weff[0:64, t, :],
                            xt[j][0:64, off : off + NFREE],
                            start=(t == 0),
                            stop=(t == NTAP - 1),
                            tile_position=(0, 0) if PAIRED else None,
                        )
                        nc.tensor.matmul(
                            psB[:],
                            weff[64:128, t, :],
                            xt[j][64:128, off : off + NFREE],
                            start=(t == 0),
                            stop=(t == NTAP - 1),
                            tile_position=(64, 0) if PAIRED else None,
                        )
                    for half, ps in ((0, psA), (1, psB)):
                        b = 2 * j + half
                        st = stage.tile([COUT, CHUNK_ROWS, OW], F32, tag="st")
                        psv = ps[:].rearrange("p (r w) -> p r w", w=W)[:, :, 0:OW]
                        nc.scalar.activation(
                            out=st[:],
                            in_=psv,
                            func=AF.Identity,
                            bias=beff[:, b : b + 1],
                            scale=1.0,
                        )
                        nc.sync.dma_start(
                            out=out[b, :, h0 : h0 + CHUNK_ROWS, :], in_=st[:]
                        )

    fix_sync_waits(nc)
    return nc


_NC = None


def _get_nc():
    global _NC
    if _NC is None:
        _NC = build()
    return _NC


def make_in_maps(inputs):
    x = np.ascontiguousarray(np.asarray(inputs["x"], dtype=np.float32))
    rvec = np.ascontiguousarray(np.asarray(inputs["routing_vector"], dtype=np.float32))
    W1 = np.ascontiguousarray(np.asarray(inputs["W1"], dtype=np.float32))
    b1 = np.ascontiguousarray(np.asarray(inputs["b1"], dtype=np.float32)).reshape(HID, 1)
    W2 = np.ascontiguousarray(np.asarray(inputs["W2"], dtype=np.float32))
    b2 = np.ascontiguousarray(np.asarray(inputs["b2"], dtype=np.float32)).reshape(EDIM, 1)
    emb = np.ascontiguousarray(np.asarray(inputs["emb"], dtype=np.float32))
    conv_w = np.asarray(inputs["conv_w"], dtype=np.float32)
    conv_b = np.ascontiguousarray(np.asarray(inputs["conv_b"], dtype=np.float32))
    # conv_w[n, co, ci, ky, kx] -> cwp[ci, n, (ky kx), co]
    cwpa = np.ascontiguousarray(
        conv_w.transpose(2, 0, 3, 4, 1).reshape(CIN, NB, NTAP, COUT)
    )
    selm = np.zeros((2, BLOC, 128), np.float32)
    for j in range(2):
        selm[j, 2 * j, 0:64] = 1.0
        selm[j, 2 * j + 1, 64:128] = 1.0
    identm = np.eye(128, dtype=np.float32)
    in_maps = []
    for c in range(NCORES):
        in_maps.append(
            {
                "x": np.ascontiguousarray(x[BLOC * c : BLOC * (c + 1)]),
                "rv": np.ascontiguousarray(rvec[BLOC * c : BLOC * (c + 1)]),
                "w1": W1,
                "bias1": b1,
                "w2": W2,
                "bias2": b2,
                "emb": emb,
                "cwp": cwpa,
                "cb": conv_b,
                "sel": selm,
                "identin": identm,
            }
        )
    return in_maps


def kernel(**inputs):
    from concourse.bass_utils import run_bass_kernel_spmd

    nc = _get_nc()
    in_maps = make_in_maps(inputs)
    res = run_bass_kernel_spmd(nc, in_maps, core_ids=list(range(NCORES)))
    return np.concatenate([r["out"] for r in res.results], axis=0)



# revision 6
# speedup vs baseline: 1.1316x; 1.1316x over previous
"""MoE routing layer on 8 Trainium2 NeuronCores (data-parallel over batch).

Per core (4 samples):
  routing MLP -> cosine sim vs embeddings -> softmax weights wf[4,10]
  w_eff[b] = sum_n wf[b,n] * conv_w[n]  (conv is linear in weights ->
  10x fewer conv FLOPs than materializing all expert convs)
  out[b] = conv2d(x[b], w_eff[b]) + b_eff[b]

Conv path runs in bf16 (x, w_eff) accumulating fp32 in PSUM. The conv is
9 shifted matmuls over the flat 58-wide grid; the PE array is quad-tiled
(2 samples on row halves x 2 chunk parities on column halves) so all four
64x64 quadrants stream concurrently. Routing operands are host-packed into
two coalesced DMA blocks; rv arrives pre-transposed so no PE transposes
are needed on the critical path.
"""
import sys

sys.path.insert(0, "/opt/trn_rl_repo")

import numpy as np
import ml_dtypes

import concourse.bass as bass
import concourse.mybir as mybir
from concourse.tile import TileContext

F32 = mybir.dt.float32
BF16 = mybir.dt.bfloat16
AF = mybir.ActivationFunctionType
ALU = mybir.AluOpType
AX = mybir.AxisListType

NCORES = 8
BLOC = 4           # samples per core
CIN = 64
COUT = 64
H = W = 58
HW = H * W         # 3364
OH = OW = 56
NB = 10            # experts
EDIM = 64
RSIZE = 512
HID = 128
NTAP = 9
GR = 7             # output rows per chunk
NGRP = 4           # 4 groups x (even chunk + odd chunk) x 7 rows = 56
NFREE = GR * W     # 406 <= 512 (one PSUM bank)
TAP_OFF = [dy * W + dx for dy in range(3) for dx in range(3)]
QUAD = True        # 4-quadrant PE tiling (else row-pair only)
NWARM_PRE = 10     # PE warmup matmuls (HAM clock-gate) before wfT/beff
NWARM_POST = 4     # and a few after wfbc to bridge the gap to conv

# blk1 column layout (128 partitions, fp32)
B1_RVT = 0                  # [128, 4c, 4b]
B1_W1 = B1_RVT + 16         # [128, 4c, 128m]
B1_W2 = B1_W1 + 512         # [128, 64]
B1_B1 = B1_W2 + 64          # [128, 1]
B1_SEL = B1_B1 + 1          # [4, 2j, 128m] (partitions 0:4)
B1_D = B1_SEL + 256         # 849

# blk2 column layout (128 partitions, fp32)
B2_EMB = 0                  # [10, 64]
B2_CB = B2_EMB + 64         # [10, 64]
B2_ID = B2_CB + 64          # [16, 16]
B2_B2 = B2_ID + 16          # [64, 1]
B2_D = B2_B2 + 1            # 145


def fix_sync_waits(nc, cap=2):
    """This walrus build allows at most `cap` sem waits per instruction.
    Splice same-engine NoOps carrying the excess waits right before any
    over-subscribed instruction (waits happen earlier => same semantics)."""
    uid = [0]
    for f in nc.m.functions:
        for blk in f.blocks:
            insts = blk.instructions  # live list
            i = 0
            while i < len(insts):
                inst = insts[i]
                si = inst.sync_info
                waits = list(si.on_wait) if si and si.on_wait else []
                icap = 1
                if len(waits) <= icap:
                    i += 1
                    continue
                keep, excess = waits[-icap:], waits[:-icap]
                for k in range(0, len(excess), icap):
                    nop = mybir.InstNoOp(
                        name=f"{inst.name}-wsplit{uid[0]}", ins=[], outs=[]
                    )
                    uid[0] += 1
                    nop.engine = inst.engine
                    nop.sync_info = mybir.SyncInfo(
                        on_wait=excess[k : k + icap], on_update=[]
                    )
                    nc.register_instruction(nop, overwrite=True)
                    insts.insert(i, nop)
                    i += 1
                inst.sync_info = mybir.SyncInfo(
                    on_wait=keep,
                    on_update=list(si.on_update) if si and si.on_update else [],
                )
                i += 1


def build():
    nc = bass.Bass(num_swdge_queues=4)
    blk1 = nc.dram_tensor("blk1", [128, B1_D], F32, kind="ExternalInput")
    blk2 = nc.dram_tensor("blk2", [128, B2_D], F32, kind="ExternalInput")
    cwp = nc.dram_tensor("cwp", [CIN, NB, NTAP, COUT], BF16, kind="ExternalInput")
    x = nc.dram_tensor("x", [BLOC, CIN, HW], BF16, kind="ExternalInput")
    out = nc.dram_tensor("out", [BLOC, COUT, OH, OW], F32, kind="ExternalOutput")

    with TileContext(nc) as tc:
        with (
            tc.tile_pool(name="consts", bufs=1) as consts,
            tc.tile_pool(name="work", bufs=2) as work,
            tc.tile_pool(name="stage", bufs=3) as stage,
            tc.tile_pool(name="ps", bufs=2, space="PSUM") as pspool,
            tc.tile_pool(name="psconv", bufs=2, space="PSUM") as psconv,
            tc.tile_pool(name="pswarm", bufs=1, space="PSUM") as pswarm,
        ):
            # ---------- DMA in: routing blocks first, then cwp, then x ----------
            b1t = consts.tile([128, B1_D], F32, tag="b1t")
            nc.sync.dma_start(out=b1t[:], in_=blk1[:])
            b2t = consts.tile([128, B2_D], F32, tag="b2t")
            nc.sync.dma_start(out=b2t[:], in_=blk2[:])
            cwp2 = consts.tile([128, NB, NTAP, COUT], BF16, tag="cwp2")
            nc.sync.dma_start(out=cwp2[0:64], in_=cwp[:])
            nc.sync.dma_start(out=cwp2[64:128], in_=cwp[:])

            xt = []
            for j in range(2):
                t = consts.tile([128, HW + 4], BF16, tag=f"xt{j}")
                nc.vector.memset(t[:, HW : HW + 4], 0.0)
                xt.append(t)
            for b in range(BLOC):
                j, half = divmod(b, 2)
                nc.gpsimd.dma_start(
                    out=xt[j][64 * half : 64 * half + 64, 0:HW], in_=x[b]
                )

            ones64 = consts.tile([EDIM, 1], F32, tag="ones64")
            nc.vector.memset(ones64[:], 1.0)

            # views into the packed blocks
            rvT = b1t[:, B1_RVT : B1_RVT + 16].rearrange("p (c b) -> p c b", c=4)
            w1sb = b1t[:, B1_W1 : B1_W1 + 512].rearrange("p (c m) -> p c m", c=4)
            w2sb = b1t[:, B1_W2 : B1_W2 + 64]
            b1sb = b1t[:, B1_B1 : B1_B1 + 1]
            selsb = b1t[0:4, B1_SEL : B1_SEL + 256].rearrange(
                "p (j m) -> p j m", j=2
            )
            embsb = b2t[0:NB, B2_EMB : B2_EMB + 64]
            cbsb = b2t[0:NB, B2_CB : B2_CB + 64]
            ident = b2t[0:16, B2_ID : B2_ID + 16]
            b2sb = b2t[0:EDIM, B2_B2 : B2_B2 + 1]

            # ---------- routing MLP (fp32; rv comes in pre-transposed) ----------
            h1 = pspool.tile([HID, BLOC], F32, tag="small")
            for c in range(4):
                nc.tensor.matmul(
                    h1[:], w1sb[:, c, :], rvT[:, c, :], start=(c == 0), stop=(c == 3)
                )
            h1r = work.tile([HID, BLOC], F32, tag="h1r")
            nc.scalar.activation(
                out=h1r[:], in_=h1[:], func=AF.Relu, bias=b1sb, scale=1.0
            )
            rps = pspool.tile([EDIM, BLOC], F32, tag="small")
            nc.tensor.matmul(rps[:], w2sb, h1r[:], start=True, stop=True)
            rsb = work.tile([EDIM, BLOC], F32, tag="rsb")
            nc.scalar.activation(
                out=rsb[:], in_=rps[:], func=AF.Identity, bias=b2sb, scale=1.0
            )

            # ---------- cosine similarity ----------
            rsq = work.tile([EDIM, BLOC], F32, tag="rsq")
            nc.vector.tensor_mul(rsq[:], rsb[:], rsb[:])
            nsq = pspool.tile([BLOC, 1], F32, tag="small")
            nc.tensor.matmul(nsq[:], rsq[:], ones64[:], start=True, stop=True)
            rln = work.tile([BLOC, 1], F32, tag="rln")
            nc.scalar.activation(out=rln[:], in_=nsq[:], func=AF.Ln)
            rinv = work.tile([BLOC, 1], F32, tag="rinv")
            nc.scalar.activation(out=rinv[:], in_=rln[:], func=AF.Exp, scale=-0.5)

            esq = work.tile([NB, EDIM], F32, tag="esq")
            nc.vector.tensor_mul(esq[:], embsb, embsb)
            ensq = work.tile([NB, 1], F32, tag="ensq")
            nc.vector.tensor_reduce(ensq[:], esq[:], axis=AX.X, op=ALU.add)
            eln = work.tile([NB, 1], F32, tag="eln")
            nc.scalar.activation(out=eln[:], in_=ensq[:], func=AF.Ln)
            einv = work.tile([NB, 1], F32, tag="einv")
            nc.scalar.activation(out=einv[:], in_=eln[:], func=AF.Exp, scale=-0.5)
            embn = work.tile([NB, EDIM], F32, tag="embn")
            nc.vector.tensor_scalar_mul(out=embn[:], in0=embsb, scalar1=einv[:])
            embnT_ps = pspool.tile([EDIM, NB], F32, tag="small")
            nc.tensor.transpose(embnT_ps[:], embn[:], ident[0:NB, 0:NB])
            embnT = work.tile([EDIM, NB], F32, tag="embnT")
            nc.scalar.copy(out=embnT[:], in_=embnT_ps[:])

            simps = pspool.tile([BLOC, NB], F32, tag="small")
            nc.tensor.matmul(simps[:], rsb[:], embnT[:], start=True, stop=True)
            sim = work.tile([BLOC, NB], F32, tag="sim")
            nc.vector.tensor_scalar_mul(out=sim[:], in0=simps[:], scalar1=rinv[:])

            # ---------- softmax ----------
            mx = work.tile([BLOC, 1], F32, tag="mx")
            nc.vector.tensor_reduce(mx[:], sim[:], axis=AX.X, op=ALU.max)
            negmx = work.tile([BLOC, 1], F32, tag="negmx")
            nc.vector.tensor_scalar_mul(out=negmx[:], in0=mx[:], scalar1=-1.0)
            ex = work.tile([BLOC, NB], F32, tag="ex")
            s = work.tile([BLOC, 1], F32, tag="s")
            nc.scalar.activation(
                out=ex[:], in_=sim[:], func=AF.Exp, bias=negmx[:], scale=1.0,
                accum_out=s[:],
            )
            sinv = work.tile([BLOC, 1], F32, tag="sinv")
            nc.vector.reciprocal(sinv[:], s[:])
            wf = work.tile([BLOC, NB], F32, tag="wf")
            nc.vector.tensor_scalar_mul(out=wf[:], in0=ex[:], scalar1=sinv[:])

            # ---------- PE warmup: keep HAM clock ungated until conv ----------
            wl = b1t[:, B1_W1 : B1_W1 + 64].bitcast(BF16)[:, 0:128]
            wr = b1t[:, B1_W1 : B1_W1 + 256].bitcast(BF16)[:, 0:512]
            warm_ps = pswarm.tile([128, 512], F32, tag="warm")
            for _ in range(NWARM_PRE):
                nc.tensor.matmul(warm_ps[:], wl, wr, start=True, stop=True)

            # ---------- effective conv bias (both partition halves) ----------
            wfT_ps = pspool.tile([NB, BLOC], F32, tag="small")
            nc.tensor.transpose(wfT_ps[:], wf[:], ident[0:BLOC, 0:BLOC])
            wfT = work.tile([NB, BLOC], F32, tag="wfT")
            nc.scalar.copy(out=wfT[:], in_=wfT_ps[:])
            beff_ps = pspool.tile([128, BLOC], F32, tag="small")
            nc.tensor.matmul(
                beff_ps[0:64], cbsb, wfT[:], start=True, stop=True,
                tile_position=(0, 0),
            )
            nc.tensor.matmul(
                beff_ps[64:128], cbsb, wfT[:], start=True, stop=True,
                tile_position=(0, 64), skip_group_check=True,
            )
            beff2 = work.tile([128, BLOC], F32, tag="beff2")
            nc.scalar.copy(out=beff2[:], in_=beff_ps[:])

            # ---------- per-pair expert weight broadcast ----------
            wfbcs = []
            for j in range(2):
                wfbc_ps = pspool.tile([128, NB], F32, tag="small")
                nc.tensor.matmul(
                    wfbc_ps[:], selsb[:, j, :], wf[:], start=True, stop=True
                )
                wfbc = work.tile([128, NB], F32, tag=f"wfbc{j}")
                nc.scalar.copy(out=wfbc[:], in_=wfbc_ps[:])
                wfbcs.append(wfbc)

            for _ in range(NWARM_POST):
                nc.tensor.matmul(warm_ps[:], wl, wr, start=True, stop=True)
            warm_sink = work.tile([1, 1], F32, tag="warm_sink")
            nc.scalar.copy(out=warm_sink[:], in_=warm_ps[0:1, 0:1])

            # ---------- w_eff chains (vector; lo half then hi half) ----------
            # weff[0:64]  = weights for sample 2j   (PE row tile 0)
            # weff[64:128] = weights for sample 2j+1 (PE row tile 64)
            weffs = []
            for j in range(2):
                weffs.append(
                    work.tile(
                        [128, NTAP, COUT], BF16, tag=f"weff{j}", name=f"weff{j}"
                    )
                )
            for half in (0, 1):
                lo, hi = 64 * half, 64 * half + 64
                for j in range(2):
                    weff, wfbc = weffs[j], wfbcs[j]
                    wv = weff[lo:hi].rearrange("p t c -> p (t c)")
                    cv = cwp2[lo:hi].rearrange("p n t c -> p n (t c)")
                    nc.vector.tensor_scalar_mul(
                        out=wv, in0=cv[:, 0, :], scalar1=wfbc[lo:hi, 0:1]
                    )
                    for n in range(1, NB):
                        nc.vector.scalar_tensor_tensor(
                            out=wv,
                            in0=cv[:, n, :],
                            scalar=wfbc[lo:hi, n : n + 1],
                            in1=wv,
                            op0=ALU.mult,
                            op1=ALU.add,
                        )

            # ---------- conv: quad-tiled 9-tap shifted matmuls ----------
            stgs = [None, None]
            for j in range(2):
                weff = weffs[j]
                for g in range(NGRP):
                    h_e = 2 * g * GR       # even chunk start row
                    h_o = h_e + GR         # odd chunk start row
                    psA = psconv.tile([128, NFREE], F32, tag="psA")
                    psB = psconv.tile([128, NFREE], F32, tag="psB")
                    for t in range(NTAP):
                        oe = h_e * W + TAP_OFF[t]
                        oo = h_o * W + TAP_OFF[t]
                        st_, sp = (t == 0), (t == NTAP - 1)
                        nc.tensor.matmul(
                            psA[0:64], weff[0:64, t, :], xt[j][0:64, oe : oe + NFREE],
                            start=st_, stop=sp, tile_position=(0, 0),
                            skip_group_check=True,
                        )
                        nc.tensor.matmul(
                            psA[64:128], weff[0:64, t, :], xt[j][0:64, oo : oo + NFREE],
                            start=st_, stop=sp, tile_position=(0, 64),
                            skip_group_check=True,
                        )
                        nc.tensor.matmul(
                            psB[0:64], weff[64:128, t, :], xt[j][64:128, oe : oe + NFREE],
                            start=st_, stop=sp, tile_position=(64, 0),
                            skip_group_check=True,
                        )
                        nc.tensor.matmul(
                            psB[64:128], weff[64:128, t, :], xt[j][64:128, oo : oo + NFREE],
                            start=st_, stop=sp, tile_position=(64, 64),
                            skip_group_check=True,
                        )
                    # evacuate: +bias, trim 58->56 cols; even half on scalar,
                    # odd half on gpsimd (keeps pace with the PE)
                    gi = g % 2
                    for half, ps in ((0, psA), (1, psB)):
                        b = 2 * j + half
                        if gi == 0:
                            stgs[half] = stage.tile(
                                [128, 2, GR, OW], F32,
                                tag=f"stg{half}", name=f"stg{half}",
                            )
                        st = stgs[half]
                        pv = ps[:].rearrange("p (r w) -> p r w", w=W)[:, :, 0:OW]
                        nc.scalar.activation(
                            out=st[0:64, gi], in_=pv[0:64], func=AF.Identity,
                            bias=beff2[0:64, b : b + 1], scale=1.0,
                        )
                        nc.vector.tensor_scalar_add(
                            out=st[64:128, gi], in0=pv[64:128],
                            scalar1=beff2[64:128, b : b + 1],
                        )
                        # after 2 groups: store both row-parities of the window
                        if gi == 1:
                            dv = out[b].rearrange(
                                "c (G g2 r) w -> g2 c G r w", G=NGRP, g2=2
                            )
                            nc.sync.dma_start(
                                out=dv[0, :, g - 1 : g + 1], in_=st[0:64]
                            )
                            nc.sync.dma_start(
                                out=dv[1, :, g - 1 : g + 1], in_=st[64:128]
                            )

    fix_sync_waits(nc)
    return nc


_NC = None


def _get_nc():
    global _NC
    if _NC is None:
        _NC = build()
    return _NC


def make_in_maps(inputs):
    bf16 = ml_dtypes.bfloat16
    x = np.asarray(inputs["x"], dtype=np.float32).reshape(32, CIN, HW)
    rvec = np.asarray(inputs["routing_vector"], dtype=np.float32)
    W1 = np.asarray(inputs["W1"], dtype=np.float32)
    b1 = np.asarray(inputs["b1"], dtype=np.float32)
    W2 = np.asarray(inputs["W2"], dtype=np.float32)
    b2 = np.asarray(inputs["b2"], dtype=np.float32)
    emb = np.asarray(inputs["emb"], dtype=np.float32)
    conv_w = np.asarray(inputs["conv_w"], dtype=np.float32)
    conv_b = np.asarray(inputs["conv_b"], dtype=np.float32)

    # conv_w[n, co, ci, ky, kx] -> cwp[ci, n, (ky kx), co], bf16
    cwpa = np.ascontiguousarray(
        conv_w.transpose(2, 0, 3, 4, 1).reshape(CIN, NB, NTAP, COUT)
    ).astype(bf16)

    # blk1: per-core rvT + shared routing weights + sel masks
    w1blk = W1.reshape(4, 128, HID).transpose(1, 0, 2).reshape(128, 512)
    selm = np.zeros((4, 2, 128), np.float32)
    for j in range(2):
        selm[2 * j, j, 0:64] = 1.0
        selm[2 * j + 1, j, 64:128] = 1.0
    blk1_shared = np.zeros((128, B1_D), np.float32)
    blk1_shared[:, B1_W1 : B1_W1 + 512] = w1blk
    blk1_shared[:, B1_W2 : B1_W2 + 64] = W2
    blk1_shared[:, B1_B1] = b1
    blk1_shared[0:4, B1_SEL : B1_SEL + 256] = selm.reshape(4, 256)

    # blk2: emb, conv bias, identity, b2
    blk2a = np.zeros((128, B2_D), np.float32)
    blk2a[0:NB, B2_EMB : B2_EMB + 64] = emb
    blk2a[0:NB, B2_CB : B2_CB + 64] = conv_b
    blk2a[0:16, B2_ID : B2_ID + 16] = np.eye(16, dtype=np.float32)
    blk2a[0:EDIM, B2_B2] = b2

    xb = x.astype(bf16)
    in_maps = []
    for c in range(NCORES):
        blk1a = blk1_shared.copy()
        rvc = rvec[BLOC * c : BLOC * (c + 1)]          # [4, 512]
        # rvT[p, c, b] = rv[b, 128c + p]
        rvt = rvc.T.reshape(4, 128, BLOC).transpose(1, 0, 2).reshape(128, 16)
        blk1a[:, B1_RVT : B1_RVT + 16] = rvt
        in_maps.append(
            {
                "blk1": blk1a,
                "blk2": blk2a,
                "cwp": cwpa,
                "x": np.ascontiguousarray(xb[BLOC * c : BLOC * (c + 1)]),
            }
        )
    return in_maps


def kernel(**inputs):
    from concourse.bass_utils import run_bass_kernel_spmd

    nc = _get_nc()
    in_maps = make_in_maps(inputs)
    res = run_bass_kernel_spmd(nc, in_maps, core_ids=list(range(NCORES)))
    return np.concatenate([r["out"] for r in res.results], axis=0)
